# revision 4
# baseline (speedup 1.0000x reference)
"""GNN message-passing (ConvGraph) Trainium2 Bass kernel, 8 NeuronCores.

Computes out = segment_sum(edge_weight * (x @ W)[edge_src], edge_dst) for a
graph with N nodes and E edges.

Strategy:
  - Shard nodes (rows of x / out) across the 8 cores; replicate W.
  - Everything except the final accumulation runs in bf16 (tolerance is
    2e-2; bf16 keeps us ~3e-3).
  - Each core computes its h shard = x_m @ W on TensorE (bf16), then an
    AllGather makes the full h table [N_pad, 128] bf16 resident in every
    core's HBM.
  - The h table is processed in NCHUNK chunks.  Each chunk (CH rows,
    bf16) is DMA'd contiguously into SBUF; the per-edge rows are then
    fetched with an SBUF-source dma_gather (256B rows, int16 chunk-local
    indices).  This avoids the random-512B-HBM-read pattern that is
    ~100x slower than SBUF-side gathering.
  - The SBUF gather emits feature-major tiles [f=128, e]; each 128-edge
    group is transposed back to edge-major on the PE (bf16 transpose via
    identity), copied PSUM->SBUF on the scalar engine, and reduced with
    one matmul per group: psum[dst128, f128] += S_T[e,dst].T @ msgs[e,f],
    where S_T = (iota == dst_local) * w is built on VectorE in a single
    fused tensor_scalar op (all bf16).
  - Per-chunk partial sums accumulate into an SBUF fp32 accumulator
    [128, NB, 128]; one final DMA writes the fp32 output.

Host-side work is limited to sharding/layout: edge partitioning +
sorting, index conversion, bf16 casts, and the x transpose/permutation
(input staging).  The x rows of each core are permuted so that the
chunk's gather layout (row idx at partition idx%128, byte offset
(idx//128)*256) is produced by a single contiguous DMA.
"""

import os
import sys
from contextlib import ExitStack

import numpy as np

for _p in ("/opt/trn_rl_repo",):
    if _p not in sys.path and os.path.isdir(_p):
        sys.path.insert(0, _p)

import ml_dtypes  # noqa: E402

import concourse.bass as bass  # noqa: E402
import concourse.mybir as mybir  # noqa: E402
import concourse.tile as tile  # noqa: E402
from concourse import bacc, library_config  # noqa: E402
from concourse.bass_utils import run_bass_kernel_spmd  # noqa: E402

N_CORES = 8
P = 128
D_IN = 256
D_OUT = 128
NCHUNK = 4  # h-table chunks (2 core shards per chunk)

BF16 = ml_dtypes.bfloat16


def make_cfg(n_nodes: int) -> dict:
    assert n_nodes % N_CORES == 0
    r0 = n_nodes // N_CORES
    r = ((r0 + P - 1) // P) * P
    nb = r // P
    sb = 1
    for cand in (7, 8, 6, 5, 4, 9, 10, 3, 2, 14, 1):
        if nb % cand == 0:
            sb = cand
            break
    ch = (N_CORES * r) // NCHUNK
    assert ch <= 32767, f"chunk rows {ch} exceed int16 index range"
    assert ch == 2 * r
    nr = ch // P  # ranks per chunk (= bytes/256 per partition)
    assert r % 64 == 0 and nr == r // 64
    return dict(
        n_nodes=n_nodes, R0=r0, R=r, NB=nb, SB=sb, NSB=nb // sb, CH=ch, NR=nr
    )


ABLATE = os.environ.get("GNN_ABLATE", "")


def build_bass(cfg: dict, S: int):
    """Build the SPMD Bass program (same NEFF for all 8 cores)."""
    R, NB, SB, NSB, CH, NR = (
        cfg["R"], cfg["NB"], cfg["SB"], cfg["NSB"], cfg["CH"], cfg["NR"]
    )
    NG = NB * NCHUNK * S  # total 128-edge groups per core
    NGC = SB * S  # groups per gather call
    NI = NGC * P  # idxs per gather call
    TOT = NG * P  # total padded edge slots per core
    f32 = mybir.dt.float32
    bf16 = mybir.dt.bfloat16
    i16 = mybir.dt.int16

    nc = bacc.Bacc(
        "TRN2",
        target_bir_lowering=False,
        debug=False,
        num_devices=N_CORES,
        num_swdge_queues=4,
    )

    xT = nc.declare_dram_parameter("xT", [D_IN, R], bf16, isOutput=False)
    Wp = nc.declare_dram_parameter("W", [D_IN, D_OUT], bf16, isOutput=False)
    iota = nc.declare_dram_parameter("iota", [P, P], bf16, isOutput=False)
    ident = nc.declare_dram_parameter("ident", [P, P], bf16, isOutput=False)
    idxp = nc.declare_dram_parameter("idx", [P, TOT // 16], i16, isOutput=False)
    wgtp = nc.declare_dram_parameter("wgt", [P, NG], f32, isOutput=False)
    dstp = nc.declare_dram_parameter("dstl", [P, NG], f32, isOutput=False)
    outp = nc.declare_dram_parameter("out", [R, D_OUT], f32, isOutput=True)

    h_shard = nc.dram_tensor("h_shard", [R, D_OUT], bf16)
    h_full = nc.dram_tensor(
        "h_full", [N_CORES * R, D_OUT], bf16, addr_space="Shared"
    )

    DK = D_IN // P  # k-chunks for the projection matmul

    with tile.TileContext(nc) as tc, ExitStack() as ctx:
        const = ctx.enter_context(tc.tile_pool(name="const", bufs=1))
        xpool = ctx.enter_context(tc.tile_pool(name="xp", bufs=2))
        hstage = ctx.enter_context(tc.tile_pool(name="hst", bufs=2))
        psum = ctx.enter_context(tc.tile_pool(name="ps", bufs=4, space="PSUM"))
        psumt = ctx.enter_context(tc.tile_pool(name="pst", bufs=4, space="PSUM"))
        hpool = ctx.enter_context(tc.tile_pool(name="htab", bufs=1))
        gpool = ctx.enter_context(tc.tile_pool(name="gat", bufs=2))
        ipool = ctx.enter_context(tc.tile_pool(name="idxp", bufs=3))
        mpool = ctx.enter_context(tc.tile_pool(name="meta", bufs=4))
        spool = ctx.enter_context(tc.tile_pool(name="oneh", bufs=6))
        mspool = ctx.enter_context(tc.tile_pool(name="msg", bufs=6))
        apool = ctx.enter_context(tc.tile_pool(name="accp", bufs=1))

        nc.gpsimd.load_library(library_config.mlp)

        w_t = const.tile([P, DK, P], bf16)
        for k in range(DK):
            nc.sync.dma_start(out=w_t[:, k, :], in_=Wp[k * P : (k + 1) * P, :])
        iota_t = const.tile([P, P], bf16)
        nc.sync.dma_start(out=iota_t[:], in_=iota[:])
        ident_t = const.tile([P, P], bf16)
        nc.sync.dma_start(out=ident_t[:], in_=ident[:])

        # Phase A: h_shard = x_m @ W (bf16)
        TS = 8  # row-tiles per strip
        nstrip = (NB + TS - 1) // TS
        for s_ in range(nstrip):
            t0 = s_ * TS
            t1 = min(NB, t0 + TS)
            nt = t1 - t0
            xk = []
            for k in range(DK):
                xkt = xpool.tile([P, TS * P], bf16, tag=f"x{k}")
                nc.sync.dma_start(
                    out=xkt[:, : nt * P],
                    in_=xT[k * P : (k + 1) * P, t0 * P : t1 * P],
                )
                xk.append(xkt)
            hst = hstage.tile([P, TS, P], bf16, tag="hst")
            for t in range(nt):
                ps = psum.tile([P, P], f32, tag="ps")
                for k in range(DK):
                    nc.tensor.matmul(
                        ps[:],
                        xk[k][:, t * P : (t + 1) * P],
                        w_t[:, k, :],
                        start=(k == 0),
                        stop=(k == DK - 1),
                    )
                nc.scalar.copy(out=hst[:, t, :], in_=ps[:])
            nc.sync.dma_start(
                out=h_shard[t0 * P : t1 * P, :].rearrange(
                    "(t p) f -> p t f", p=P
                ),
                in_=hst[:, :nt, :],
            )

        # Phase B: AllGather h across the 8 cores (bf16)
        nc.gpsimd.collective_compute(
            "AllGather",
            mybir.AluOpType.bypass,
            ins=[h_shard[:]],
            outs=[h_full[:]],
            replica_groups=[list(range(N_CORES))],
        )

        # Phase C: per-chunk SBUF gather + weighted segment-sum
        acc = apool.tile([P, NB, P], f32)
        for c in range(NCHUNK if "nophasec" not in ABLATE else 0):
            # Chunk c of the h table -> SBUF, gather layout: row idx at
            # partition idx%128, byte offset (idx//128)*256.  The host
            # permutation makes this a contiguous per-partition DMA.
            htab = hpool.tile([P, NR * P], bf16, tag="htab")
            nc.sync.dma_start(
                out=htab[:],
                in_=h_full[c * CH : (c + 1) * CH, :].rearrange(
                    "(p s) f -> p (s f)", p=P
                ),
            )
            for sb in range(NSB):
                call = c * NSB + sb
                it = ipool.tile([P, NI // 16], i16, tag="it")
                nc.sync.dma_start(
                    out=it[:],
                    in_=idxp[:, call * (NI // 16) : (call + 1) * (NI // 16)],
                )
                gt = gpool.tile([P, 1, NI], bf16, tag="gt")
                if "nogather" in ABLATE:
                    nc.vector.memset(gt[:], 0.0)
                else:
                    nc.gpsimd.dma_gather(
                        gt[:],
                        htab[:],
                        it[:],
                        NI,
                        NI,
                        P,
                        transpose=True,
                        single_packet=False,
                        queue_num=call % 4,
                        sbuf_tokens_per_rank=P,
                        sbuf_free_dim_per_rank=256,
                        sbuf_free_dim_pad_per_rank=0,
                        sbuf_byte_offset=0,
                    )
                wt = mpool.tile([P, NGC], f32, tag="wt")
                dt = mpool.tile([P, NGC], f32, tag="dt")
                nc.sync.dma_start(
                    out=wt[:], in_=wgtp[:, call * NGC : (call + 1) * NGC]
                )
                nc.sync.dma_start(
                    out=dt[:], in_=dstp[:, call * NGC : (call + 1) * NGC]
                )
                for b in range(SB):
                    ps = psum.tile([P, P], f32, tag="ps", name=f"psb_{call}_{b}")
                    for j in range(S):
                        g = b * S + j
                        # Transpose the feature-major gather block back to
                        # edge-major on the PE (bf16 transpose).
                        tps = psumt.tile([P, P], bf16, tag="tps")
                        nc.tensor.transpose(
                            tps[:], gt[:, 0, g * P : (g + 1) * P], ident_t[:]
                        )
                        ms = mspool.tile([P, P], bf16, tag="ms")
                        nc.scalar.copy(out=ms[:], in_=tps[:])
                        st = spool.tile([P, P], bf16, tag="st")
                        nc.vector.tensor_scalar(
                            out=st[:],
                            in0=iota_t[:],
                            scalar1=dt[:, g : g + 1],
                            scalar2=wt[:, g : g + 1],
                            op0=mybir.AluOpType.is_equal,
                            op1=mybir.AluOpType.mult,
                        )
                        nc.tensor.matmul(
                            ps[:],
                            st[:],
                            ms[:],
                            start=(j == 0),
                            stop=(j == S - 1),
                        )
                    blk = sb * SB + b
                    if c == 0:
                        nc.scalar.copy(out=acc[:, blk, :], in_=ps[:])
                    else:
                        nc.vector.tensor_tensor(
                            out=acc[:, blk, :],
                            in0=acc[:, blk, :],
                            in1=ps[:],
                            op=mybir.AluOpType.add,
                        )
        if "nophasec" in ABLATE:
            nc.vector.memset(acc[:], 0.0)
        nc.sync.dma_start(
            out=outp[:].rearrange("(b p) f -> p b f", p=P),
            in_=acc[:],
        )

    nc.compile()
    return nc


def host_prep(x, W, edge_src, edge_dst, edge_weight, cfg):
    """Shard + stage inputs. Returns (in_maps, S)."""
    R0, R, NB, SB, NSB, CH, NR = (
        cfg["R0"], cfg["R"], cfg["NB"], cfg["SB"], cfg["NSB"], cfg["CH"],
        cfg["NR"],
    )
    x = np.asarray(x, dtype=np.float32)
    W = np.asarray(W, dtype=np.float32)
    edge_src = np.asarray(edge_src, dtype=np.int64)
    edge_dst = np.asarray(edge_dst, dtype=np.int64)
    edge_weight = np.asarray(edge_weight, dtype=np.float32)

    # Source-node chunk-local index: src core m_s = n // R0, local l;
    # chunk c = m_s // 2; idx16 = (l//64)*128 + (m_s%2)*64 + (l%64).
    m_s = edge_src // R0
    l_s = edge_src - m_s * R0
    src_chunk = (m_s // 2).astype(np.int64)
    idx16_all = ((l_s // 64) * P + (m_s % 2) * 64 + (l_s % 64)).astype(np.int64)

    core_of = edge_dst // R0
    per_core = []
    max_count = 1
    for m in range(N_CORES):
        sel = core_of == m
        d = edge_dst[sel] - m * R0
        w = edge_weight[sel]
        b = d // P
        dstl = (d % P).astype(np.float32)
        c = src_chunk[sel]
        lidx = idx16_all[sel].astype(np.int16)
        key = (b * NCHUNK + c).astype(np.int64)
        counts = np.bincount(key, minlength=NB * NCHUNK)
        max_count = max(max_count, int(counts.max()))
        per_core.append((b, c, dstl, lidx, w, key, counts))

    S = (max_count + P - 1) // P
    NG = NB * NCHUNK * S
    TOT = NG * P

    iota_np = np.tile(
        np.arange(P, dtype=np.float32)[None, :], (P, 1)
    ).astype(BF16)
    ident_np = np.eye(P, dtype=np.float32).astype(BF16)

    in_maps = []
    for m in range(N_CORES):
        b, c, dstl, lidx, w, key, counts = per_core[m]
        # Device loop order: chunk-major, then dst superblock, then block
        # within superblock, then the S groups of 128 slots.  Sort edges
        # by (chunk, block) bucket, then by gather idx within each bucket.
        order = np.lexsort((lidx, key))
        key_s = key[order]
        starts = np.zeros(NB * NCHUNK + 1, dtype=np.int64)
        np.cumsum(counts, out=starts[1:])
        rank = np.arange(len(key_s)) - starts[key_s]
        bb = b[order]
        cc = c[order]
        slot_base = (
            (cc * NSB + bb // SB) * (SB * S) + (bb % SB) * S
        ) * P
        slot = slot_base + rank

        idx_stream = np.zeros(TOT, dtype=np.int16)
        wgt_stream = np.zeros(TOT, dtype=np.float32)
        dst_stream = np.zeros(TOT, dtype=np.float32)
        idx_stream[slot] = lidx[order]
        wgt_stream[slot] = w[order]
        dst_stream[slot] = dstl[order]

        idx_wrapped = np.ascontiguousarray(
            np.tile(idx_stream.reshape(-1, 16).T, (8, 1))
        )
        wgt_tile = np.ascontiguousarray(wgt_stream.reshape(NG, P).T)
        dst_tile = np.ascontiguousarray(dst_stream.reshape(NG, P).T)

        # x staging: node with shard-local id l goes to staged row
        # p_loc = (l%64)*NR + l//64, so that the device's contiguous
        # chunk DMA lands each row at its gather slot.
        x_m = np.zeros((R, D_IN), dtype=BF16)
        l_ids = np.arange(R0, dtype=np.int64)
        p_loc = (l_ids % 64) * NR + l_ids // 64
        x_m[p_loc] = x[m * R0 : (m + 1) * R0].astype(BF16)
        xT_m = np.ascontiguousarray(x_m.T)

        in_maps.append(
            {
                "xT": xT_m,
                "W": W.astype(BF16),
                "iota": iota_np,
                "ident": ident_np,
                "idx": idx_wrapped,
                "wgt": wgt_tile,
                "dstl": dst_tile,
            }
        )
    return in_maps, S


_BUILD_CACHE: dict = {}


def run(x, W, edge_src, edge_dst, edge_weight, trace=False, trace_kwargs=None):
    n_nodes = x.shape[0]
    cfg = make_cfg(n_nodes)
    in_maps, S = host_prep(x, W, edge_src, edge_dst, edge_weight, cfg)
    key = (n_nodes, S)
    if key not in _BUILD_CACHE:
        _BUILD_CACHE[key] = build_bass(cfg, S)
    nc = _BUILD_CACHE[key]
    res = run_bass_kernel_spmd(
        nc,
        in_maps,
        core_ids=list(range(N_CORES)),
        trace=trace,
        **(trace_kwargs or {}),
    )
    R0, R = cfg["R0"], cfg["R"]
    out = np.concatenate(
        [np.asarray(res.results[m]["out"])[:R0] for m in range(N_CORES)], axis=0
    )
    return out, res


def kernel(**inputs) -> np.ndarray:
    out, _ = run(
        inputs["x"],
        inputs["W"],
        inputs["edge_src"],
        inputs["edge_dst"],
        inputs["edge_weight"],
        trace=False,
    )
    return out
